# revision 28
# baseline (speedup 1.0000x reference)
"""EntityAggrNet (2-layer GNN message passing) on 8 Trainium2 NeuronCores.

Strategy
--------
Node-parallel sharding: core w owns nodes [w*2048, (w+1)*2048).  Edges are
sorted by src on the host; each core processes the edges whose src lands in
its node range (edge counts per core are within ~1% of E/8 for random edges).

Per layer, per core:
  1. dma_gather x[dst] rows (fp32r, 1KB rows) from a replicated HBM copy of
     the layer input, 2048 edges per gather call.
  2. Segment-sum via one-hot selector matmuls: for each 128-edge chunk,
     selector[p, j] = (src[p] - window_base == j) built with a DVE is_equal
     against an iota row; PE accumulates selector.T @ gathered into a PSUM
     window of 128 nodes.  Mean = PSUM * (1/cnt) on evacuation.
  3. Linearity moves the weight matmuls out of the edge loop:
     mean(x[dst]) @ W_msg.  The edge-feature path collapses to
     Hn @ (emb_table @ W_edge) where Hn[n, d] = count(src=n, feat=d)/cnt[n]
     is a host-computed *index* matrix; biases ride along as extra Hn rows.
  4. Dense phase in feature-major layout (PE transposes), BatchNorm stats
     via a 2KB AllReduce, scale+shift+ReLU fused on the scalar engine.
  5. Layer output is transposed back to node-major and AllGathered (fp32r)
     to become the next layer's gather source.

All matmuls run as float32r (TRN2 full-rate fp32, ~1e-4 rounding).
"""
import os
import sys

if "/opt/trn_rl_repo" not in sys.path:
    sys.path.insert(0, "/opt/trn_rl_repo")

import numpy as np

import concourse.bass as bass  # noqa: F401  (engine types referenced via nc)
import concourse.tile as tile
from concourse import bacc, mybir
from concourse import bass_utils
from concourse.bass_interp import get_hw_module

F32 = mybir.dt.float32
F32R = mybir.dt.float32r
I16 = mybir.dt.int16
BF16 = mybir.dt.bfloat16
ALU = mybir.AluOpType
ACTF = mybir.ActivationFunctionType

EPS = 1e-5
B, S, D = 32, 512, 256
N = B * S                # 16384 nodes
DS, DD = 64, 64          # embedding table: [DS, DD]
L = 2                    # layers
NCORE = 8
NPC = N // NCORE         # 2048 nodes per core
WIN = 128                # nodes per PSUM window
NWIN = NPC // WIN        # 16 windows per core
NWING = N // WIN         # 128 windows globally
PIECE = 512              # edges per dma_gather call (4 chunks)

_CACHE = {}


def _build(nch):
    """Build + schedule + bacc-compile the SPMD program.

    nch: chunks (of 128 edges) per 128-node window, uniform across cores
    (host pads every window to nch*128 edges).
    """
    cap = nch * WIN                  # padded edges per window
    epc = NWIN * cap                 # padded edges per core
    npiece = epc // PIECE            # gather calls per layer
    assert epc % PIECE == 0

    nc = bacc.Bacc("TRN2", target_bir_lowering=False, debug=False,
                   num_devices=NCORE, num_swdge_queues=4)

    # ---- I/O ----
    x0 = nc.dram_tensor("x0", [N, D], BF16, kind="ExternalInput")
    xT0 = nc.dram_tensor("xT0", [D, NPC], F32R, kind="ExternalInput")
    idx_in = nc.dram_tensor("idx_in", [128, epc // 16], I16, kind="ExternalInput")
    srcmod_in = nc.dram_tensor("srcmod_in", [128, NWIN * nch], BF16, kind="ExternalInput")
    recip_in = nc.dram_tensor("recip_in", [128, NWIN], F32, kind="ExternalInput")
    hnt_in = nc.dram_tensor("hnt_in", [128, NPC], F32R, kind="ExternalInput")
    iota_in = nc.dram_tensor("iota_in", [128, 128], BF16, kind="ExternalInput")
    ident_in = nc.dram_tensor("ident_in", [128, 128], F32, kind="ExternalInput")
    wm_in = [nc.dram_tensor(f"wm{l}", [D, D], F32R, kind="ExternalInput") for l in range(L)]
    ws_in = [nc.dram_tensor(f"ws{l}", [D, D], F32R, kind="ExternalInput") for l in range(L)]
    embT_in = nc.dram_tensor("embT", [DD, DS], F32R, kind="ExternalInput")
    we_in = [nc.dram_tensor(f"we{l}", [DD, D], F32R, kind="ExternalInput") for l in range(L)]
    # rows DD..127 of the EW lhsT: [bm, be, bs, zeros...] packed on host
    ewc_in = [nc.dram_tensor(f"ewc{l}", [128 - DD, D], F32R, kind="ExternalInput")
              for l in range(L)]
    gam_in = [nc.dram_tensor(f"gam{l}", [D, 1], F32, kind="ExternalInput") for l in range(L)]
    bet_in = [nc.dram_tensor(f"bet{l}", [D, 1], F32, kind="ExternalInput") for l in range(L)]
    gbrow_in = [nc.dram_tensor(f"gbrow{l}", [4, 128], F32, kind="ExternalInput")
                for l in range(L)]
    out_ext = nc.dram_tensor("out", [NPC, D], F32, kind="ExternalOutput")

    with tile.TileContext(nc) as tc:
        with tc.tile_pool(name="const", bufs=1) as cp, \
             tc.tile_pool(name="gat", bufs=12) as gp, \
             tc.tile_pool(name="selp", bufs=12) as sp, \
             tc.tile_pool(name="xmaj", bufs=1) as xp, \
             tc.tile_pool(name="psE", bufs=4, space="PSUM") as psE, \
             tc.tile_pool(name="psT", bufs=2, space="PSUM") as psT, \
             tc.tile_pool(name="psD", bufs=2, space="PSUM") as psD, \
             tc.tile_pool(name="dram", bufs=1, space="DRAM") as dp:

            # ---- constants into SBUF ----
            NSPLIT = 16 * (PIECE // 16)
            idx_a = cp.tile([128, NSPLIT], I16)
            idx_b = cp.tile([128, epc // 16 - NSPLIT], I16)
            srcmod = cp.tile([128, NWIN * nch], BF16)
            recip = cp.tile([128, NWIN], F32)
            hnt = cp.tile([128, NPC], F32R)
            iota = cp.tile([128, 128], BF16)
            ident = cp.tile([128, 128], F32)
            ones1 = cp.tile([1, 128], F32)
            nc.vector.memset(ones1[:, :], 1.0)
            # split the idx preload so the first gathers start early
            nc.sync.dma_start(out=idx_a[:, :], in_=idx_in[:, 0:NSPLIT])
            nc.sync.dma_start(out=idx_b[:, :], in_=idx_in[:, NSPLIT:])
            nc.sync.dma_start(out=srcmod[:, :], in_=srcmod_in[:, :])
            nc.sync.dma_start(out=recip[:, :], in_=recip_in[:, :])
            nc.sync.dma_start(out=hnt[:, :], in_=hnt_in[:, :])
            nc.sync.dma_start(out=iota[:, :], in_=iota_in[:, :])
            nc.sync.dma_start(out=ident[:, :], in_=ident_in[:, :])

            wm_sb, ws_sb, ew_sb = [], [], []
            embT_sb = cp.tile([DD, DS], F32R)
            nc.sync.dma_start(out=embT_sb[:, :], in_=embT_in[:, :])
            for l in range(L):
                wm = cp.tile([128, 2, D], F32R, name=f"wm_sb{l}")
                ws = cp.tile([128, 2, D], F32R, name=f"ws_sb{l}")
                for kt in range(2):
                    nc.sync.dma_start(out=wm[:, kt, :], in_=wm_in[l][kt * 128:(kt + 1) * 128, :])
                    nc.sync.dma_start(out=ws[:, kt, :], in_=ws_in[l][kt * 128:(kt + 1) * 128, :])
                wm_sb.append(wm)
                ws_sb.append(ws)

                we = cp.tile([DD, D], F32R, name=f"we_sb{l}")
                nc.sync.dma_start(out=we[:, :], in_=we_in[l][:, :])
                psew = psT.tile([DD, D], F32, tag="pst", name=f"psew{l}")
                nc.tensor.matmul(psew[:, :], embT_sb[:, :], we[:, :], start=True, stop=True)
                ew = cp.tile([128, D], F32R, name=f"ew_sb{l}")
                nc.vector.tensor_copy(ew[0:DD, :], psew[:, :])
                nc.sync.dma_start(out=ew[DD:128, :], in_=ewc_in[l][:, :])
                ew_sb.append(ew)

            gb_sb = []  # [128, 2] gamma / beta per layer, packed per feat-half
            for l in range(L):
                gam = cp.tile([128, 2], F32, name=f"gam_sb{l}")
                bet = cp.tile([128, 2], F32, name=f"bet_sb{l}")
                for f in range(2):
                    nc.sync.dma_start(out=gam[:, f:f + 1], in_=gam_in[l][f * 128:(f + 1) * 128, :])
                    nc.sync.dma_start(out=bet[:, f:f + 1], in_=bet_in[l][f * 128:(f + 1) * 128, :])
                gb_sb.append((gam, bet))


            gbrow_sb = []
            for l in range(L):
                gbr = cp.tile([1, 4, 128], F32, name=f"gbrow_sb{l}")
                nc.sync.dma_start(out=gbr[0:1, :, :], in_=gbrow_in[l][:, :])
                gbrow_sb.append(gbr)

            xT_cur = [cp.tile([128, NPC], F32R, name=f"xT0_sb{f}") for f in range(2)]
            for f in range(2):
                nc.sync.dma_start(out=xT_cur[f][:, :], in_=xT0[f * 128:(f + 1) * 128, :])

            xsrc = x0  # gather source (DRAM AP-able handle)

            for l in range(L):
                # ================= edge phase =================
                pieces = [None] * npiece

                def ensure_piece(p, l=l, pieces=pieces):
                    if pieces[p] is not None:
                        return pieces[p]
                    g = gp.tile([128, PIECE // 128, D], BF16, tag="g", name=f"g{l}_{p}")
                    nc.gpsimd.dma_gather(
                        out_ap=g[:, :, :],
                        in_ap=xsrc[:, :],
                        idxs_ap=(idx_a[:, p * (PIECE // 16):(p + 1) * (PIECE // 16)]
                                 if p < 16 else
                                 idx_b[:, (p - 16) * (PIECE // 16):(p - 15) * (PIECE // 16)]),
                        num_idxs=PIECE, num_idxs_reg=PIECE,
                        elem_size=D, single_packet=True,
                        queue_num=p % 4)
                    s = sp.tile([128, PIECE // 128, 128], BF16, tag="s", name=f"s{l}_{p}")
                    npc_ = PIECE // 128
                    nc.vector.tensor_tensor(
                        s[:, :, :],
                        iota[:, :].unsqueeze(1).to_broadcast((128, npc_, 128)),
                        srcmod[:, p * npc_:(p + 1) * npc_].unsqueeze(2)
                              .to_broadcast((128, npc_, 128)),
                        ALU.is_equal)
                    pieces[p] = (g, s)
                    return pieces[p]

                # Fused edge + dense pipeline: windows stream through; after
                # every 4th window the corresponding 512-node dense block,
                # its stat partials, and its node-major transposes fire, so
                # by the time the last edge matmul lands almost everything
                # downstream has already drained.
                msx = xp.tile([128, NWIN, D], F32, tag="msx", name=f"msx{l}")
                preout = [xp.tile([128, NPC], F32, tag=f"pre{f}", name=f"pre{l}_{f}")
                          for f in range(2)]
                xraw = xp.tile([128, NWIN, D], F32, tag="xraw", name=f"xraw{l}")
                redp = cp.tile([128, 16], F32, tag="redp", bufs=2, name=f"redp{l}")
                sqscr = xp.tile([128, 512], F32, tag="sqscr", name=f"sqscr{l}")
                msxTn = [[None] * (NPC // 512) for _ in range(2)]
                for w in range(NWIN):
                    ps = psE.tile([128, D], F32, tag="pse", name=f"pse{l}_{w}")
                    for c in range(nch):
                        gc = w * nch + c
                        g, s = ensure_piece(gc // (PIECE // 128))
                        lc = gc % (PIECE // 128)
                        nc.tensor.matmul(ps[:, :], s[:, lc, :], g[:, lc, :],
                                         start=(c == 0), stop=(c == nch - 1))
                    nc.vector.tensor_scalar(msx[:, w, :], ps[:, :],
                                            recip[:, w:w + 1], None, ALU.mult)
                    nb, wi = w // 4, w % 4
                    for f in range(2):
                        if wi == 0:
                            msxTn[f][nb] = xp.tile([128, 512], F32R, tag=f"msxT{f}",
                                                   bufs=4, name=f"msxT{l}_{f}_{nb}")
                        pt = psT.tile([128, 128], F32, tag="pst", name=f"ptm{l}_{w}_{f}")
                        nc.tensor.transpose(pt[:, :], msx[:, w, f * 128:(f + 1) * 128],
                                            ident[:, :])
                        nc.vector.tensor_copy(msxTn[f][nb][:, wi * 128:(wi + 1) * 128],
                                              pt[:, :])
                    if wi != 3:
                        continue
                    # dense block for this group of 4 windows
                    cols = slice(nb * 512, (nb + 1) * 512)
                    for f in range(2):
                        pd = psD.tile([128, 512], F32, tag="psd", name=f"pd{l}_{f}_{nb}")
                        fo = slice(f * 128, (f + 1) * 128)
                        nc.tensor.matmul(pd[:, :], wm_sb[l][:, 0, fo], msxTn[0][nb][:, :],
                                         start=True, stop=False)
                        nc.tensor.matmul(pd[:, :], wm_sb[l][:, 1, fo], msxTn[1][nb][:, :],
                                         start=False, stop=False)
                        nc.tensor.matmul(pd[:, :], ws_sb[l][:, 0, fo], xT_cur[0][:, cols],
                                         start=False, stop=False)
                        nc.tensor.matmul(pd[:, :], ws_sb[l][:, 1, fo], xT_cur[1][:, cols],
                                         start=False, stop=False)
                        nc.tensor.matmul(pd[:, :], ew_sb[l][:, fo], hnt[:, cols],
                                         start=False, stop=True)
                        # evacuate + free per-block column sums
                        nc.vector.tensor_scalar(preout[f][:, cols], pd[:, :],
                                                1.0, 0.0, ALU.mult, ALU.add,
                                                accum_out=redp[:, f * 4 + nb:f * 4 + nb + 1])
                        # per-block sum of squares on the scalar engine
                        nc.scalar.activation(sqscr[:, :], preout[f][:, cols],
                                             ACTF.Square, bias=0.0, scale=1.0,
                                             accum_out=redp[:, 8 + f * 4 + nb:
                                                            9 + f * 4 + nb])
                    # node-major transposes of this block's pre-BN output
                    for w2 in range(nb * 4, nb * 4 + 4):
                        for f in range(2):
                            pt = psT.tile([128, 128], F32, tag="pst",
                                          name=f"ptx{l}_{w2}_{f}")
                            nc.tensor.transpose(pt[:, :],
                                                preout[f][:, w2 * 128:(w2 + 1) * 128],
                                                ident[:, :])
                            nc.vector.tensor_copy(xraw[:, w2, f * 128:(f + 1) * 128],
                                                  pt[:, :])

                # ================= batchnorm stats =================
                red = cp.tile([128, 4], F32, tag="red", bufs=2, name=f"red{l}")
                for f in range(2):
                    nc.vector.tensor_reduce(red[:, f:f + 1], redp[:, f * 4:(f + 1) * 4],
                                            mybir.AxisListType.X, ALU.add)
                    nc.vector.tensor_reduce(red[:, 2 + f:3 + f],
                                            redp[:, 8 + f * 4:8 + (f + 1) * 4],
                                            mybir.AxisListType.X, ALU.add)

                st_in = dp.tile([128, 4], F32, name=f"st_in{l}")
                st_out = dp.tile([128, 4], F32, addr_space="Shared", name=f"st_out{l}")
                nc.sync.dma_start(out=st_in[:, :], in_=red[:, :])
                nc.gpsimd.collective_compute(
                    "AllReduce", ALU.add,
                    replica_groups=[list(range(NCORE))],
                    ins=[st_in[:, :]], outs=[st_out[:, :]])
                red2 = cp.tile([128, 4], F32, tag="red", bufs=2, name=f"red2{l}")
                nc.sync.dma_start(out=red2[:, :], in_=st_out[:, :])

                # mu/var -> scale/shift  (all [128, 2])
                mo = cp.tile([128, 12], F32, tag="mo", bufs=2, name=f"mo{l}")
                mu, ex2, var, vare, sd, rsq = (mo[:, 0:2], mo[:, 2:4], mo[:, 4:6],
                                               mo[:, 6:8], mo[:, 8:10], mo[:, 10:12])
                nc.vector.tensor_scalar(mu, red2[:, 0:2], 1.0 / N, None, ALU.mult)
                nc.vector.tensor_scalar(ex2, red2[:, 2:4], 1.0 / N, None, ALU.mult)
                nc.vector.tensor_tensor(var, mu, mu, ALU.mult)
                nc.vector.tensor_tensor(var, ex2, var, ALU.subtract)
                nc.vector.tensor_scalar(vare, var, EPS, None, ALU.add)
                nc.scalar.activation(sd, vare, ACTF.Sqrt, bias=0.0, scale=1.0)
                nc.vector.reciprocal(rsq, sd)
                gam, bet = gb_sb[l]
                sc = cp.tile([128, 4], F32, tag="sc", bufs=2, name=f"sc{l}")
                scale2, shift2 = sc[:, 0:2], sc[:, 2:4]
                nc.vector.tensor_tensor(scale2, gam[:, :], rsq, ALU.mult)
                nc.vector.tensor_tensor(shift2, mu, scale2, ALU.mult)
                nc.vector.tensor_tensor(shift2, bet[:, :], shift2, ALU.subtract)

                # ===== broadcast scale/shift along partitions (node-major BN) =====
                # row-form stats straight from the AllReduce output in DRAM
                redrow = cp.tile([1, 4, 128], F32, tag="redrow", bufs=2, name=f"redrow{l}")
                nc.sync.dma_start(out=redrow[0:1, :, :], in_=st_out[:, :].transpose([1, 0]))
                morow = cp.tile([1, 14, 128], F32, tag="morow", bufs=2, name=f"morow{l}")
                mu_r = morow[0:1, 0:2, :]
                ex2_r = morow[0:1, 2:4, :]
                var_r = morow[0:1, 4:6, :]
                vare_r = morow[0:1, 6:8, :]
                sd_r = morow[0:1, 8:10, :]
                rsq_r = morow[0:1, 10:12, :]
                nc.vector.tensor_scalar(mu_r, redrow[0:1, 0:2, :], 1.0 / N, None, ALU.mult)
                nc.vector.tensor_scalar(ex2_r, redrow[0:1, 2:4, :], 1.0 / N, None, ALU.mult)
                nc.vector.tensor_tensor(var_r, mu_r, mu_r, ALU.mult)
                nc.vector.tensor_tensor(var_r, ex2_r, var_r, ALU.subtract)
                nc.vector.tensor_scalar(vare_r, var_r, EPS, None, ALU.add)
                nc.scalar.activation(sd_r, vare_r, ACTF.Sqrt, bias=0.0, scale=1.0)
                nc.vector.reciprocal(rsq_r, sd_r)
                gbrow = gbrow_sb[l]  # [1, 4, 128]: gamma rows 0:2, beta rows 2:4
                scrow = cp.tile([1, 4, 128], F32, tag="scrow", bufs=2, name=f"scrow{l}")
                nc.vector.tensor_tensor(scrow[0:1, 0:2, :], gbrow[0:1, 0:2, :], rsq_r,
                                        ALU.mult)
                nc.vector.tensor_tensor(scrow[0:1, 2:4, :], mu_r, scrow[0:1, 0:2, :],
                                        ALU.mult)
                nc.vector.tensor_tensor(scrow[0:1, 2:4, :], gbrow[0:1, 2:4, :],
                                        scrow[0:1, 2:4, :], ALU.subtract)
                scb = cp.tile([128, 2, D], F32, tag="scb", bufs=2, name=f"scb{l}")
                for j in range(2):  # j=0: scale, j=1: shift
                    pb = psT.tile([128, D], F32, tag="pst", name=f"pb{l}_{j}")
                    nc.tensor.matmul(pb[:, 0:128], ones1[:, :], scrow[0:1, 2 * j, :],
                                     start=True, stop=False)
                    nc.tensor.matmul(pb[:, 128:256], ones1[:, :], scrow[0:1, 2 * j + 1, :],
                                     start=False, stop=True)
                    nc.vector.tensor_copy(scb[:, j, :], pb[:, :])

                # ===== apply BN + ReLU node-major (batched); ship out =====
                if l < L - 1:
                    agi = dp.tile([NPC, D], BF16, name=f"agi{l}")
                    ago = dp.tile([N, D], BF16, addr_space="Shared", name=f"ago{l}")
                tmp = xp.tile([128, NWIN, D], F32, tag="msx", name=f"xtmp{l}")
                xrow_all = xp.tile([128, NWIN, D], F32 if l == L - 1 else BF16,
                                   tag="msx" if l == L - 1 else "xrowall",
                                   name=f"xrow{l}")
                nc.vector.scalar_tensor_tensor(
                    tmp[:, :, :], xraw[:, :, :], 1.0,
                    scb[:, 0, :].unsqueeze(1).to_broadcast((128, NWIN, D)),
                    ALU.mult, ALU.mult)
                nc.vector.tensor_tensor(
                    xraw[:, :, :], tmp[:, :, :],
                    scb[:, 1, :].unsqueeze(1).to_broadcast((128, NWIN, D)), ALU.add)
                nc.vector.tensor_scalar(xrow_all[:, :, :], xraw[:, :, :], 0.0, None,
                                        ALU.max)
                dst = agi if l < L - 1 else out_ext
                dst_ap = dst[:, :].rearrange("(w p) d -> p w d", p=128)
                nc.sync.dma_start(out=dst_ap, in_=xrow_all[:, :, :])

                if l < L - 1:
                    # feat-major post-BN activations for the next layer's self path
                    xnT = [xp.tile([128, NPC], F32R, tag=f"xnT{f}", name=f"xnT{l}_{f}")
                           for f in range(2)]
                    for f in range(2):
                        nc.scalar.activation(xnT[f][:, :], preout[f][:, :], ACTF.Relu,
                                             bias=shift2[:, f:f + 1],
                                             scale=scale2[:, f:f + 1])
                    nc.gpsimd.collective_compute(
                        "AllGather", ALU.bypass,
                        replica_groups=[list(range(NCORE))],
                        ins=[agi[:, :]], outs=[ago[:, :]])
                    xsrc = ago
                    xT_cur = xnT

    nc.compile()
    nc.m = get_hw_module(nc.m)
    return nc


def _preprocess(data, edge, edge_feature):
    """Host-side index preprocessing: sort edges by src, window-pad, build
    count matrices.  Touches only index arrays (+ dtype/layout of data)."""
    src = np.asarray(edge[0], dtype=np.int64)
    dst = np.asarray(edge[1], dtype=np.int64)
    ef = np.asarray(edge_feature, dtype=np.int64)

    order = np.argsort(src, kind="stable")
    src_s = src[order]
    dst_s = dst[order]

    cnt = np.bincount(src, minlength=N)
    recip = (1.0 / np.maximum(cnt, 1)).astype(np.float32)
    H = np.bincount(src * DS + ef, minlength=N * DS).reshape(N, DS)
    Hn = (H * recip[:, None]).astype(np.float32)

    wid = src_s // WIN
    wcnt = np.bincount(wid, minlength=NWING)
    nch = max(int(np.ceil(wcnt.max() / 128)), 1)
    cap = nch * WIN

    wstart = np.zeros(NWING + 1, np.int64)
    np.cumsum(wcnt, out=wstart[1:])
    idx_pad = np.zeros((NWING, cap), np.int16)
    srm_pad = np.full((NWING, cap), -1.0, np.float32)
    for g in range(NWING):
        a, b = wstart[g], wstart[g + 1]
        k = b - a
        idx_pad[g, :k] = dst_s[a:b].astype(np.int16)
        srm_pad[g, :k] = (src_s[a:b] - g * WIN).astype(np.float32)

    per_core = []
    for w in range(NCORE):
        gsl = slice(w * NWIN, (w + 1) * NWIN)
        nsl = slice(w * NPC, (w + 1) * NPC)
        flat_idx = idx_pad[gsl].reshape(-1)           # [NWIN*cap]
        idx_tile = np.tile(flat_idx.reshape(-1, 16).T, (8, 1)).astype(np.int16)
        srcmod = srm_pad[gsl].reshape(-1, 128).T.copy()      # [128, NWIN*nch]
        recip_sw = recip[nsl].reshape(NWIN, 128).T.copy()    # [128, NWIN]
        hnt = np.zeros((128, NPC), np.float32)
        hnt[:DS, :] = Hn[nsl].T
        nz = (cnt[nsl] > 0).astype(np.float32)
        hnt[DS, :] = nz
        hnt[DS + 1, :] = nz
        hnt[DS + 2, :] = 1.0
        xT0 = np.ascontiguousarray(
            data.reshape(N, D)[nsl].T.astype(np.float32))
        import ml_dtypes as _md
        per_core.append(dict(idx_in=idx_tile, srcmod_in=srcmod.astype(_md.bfloat16),
                             recip_in=recip_sw, hnt_in=hnt, xT0=xT0))
    return nch, per_core


def kernel(data, emb_table, W_msg, b_msg, W_self, b_self, W_edge, b_edge,
           bn_gamma, bn_beta, edge, edge_feature):
    data = np.asarray(data)
    nch, per_core = _preprocess(data, np.asarray(edge), np.asarray(edge_feature))

    if nch not in _CACHE:
        _CACHE[nch] = _build(nch)
    nc = _CACHE[nch]

    import ml_dtypes
    x0 = np.ascontiguousarray(data.reshape(N, D).astype(ml_dtypes.bfloat16))
    iota = np.broadcast_to(np.arange(128), (128, 128)).astype(ml_dtypes.bfloat16)
    ident = np.eye(128, dtype=np.float32)
    common = {
        "x0": x0, "iota_in": iota, "ident_in": ident,
        "embT": np.ascontiguousarray(np.asarray(emb_table, np.float32).T),
    }
    for l in range(L):
        common[f"wm{l}"] = np.ascontiguousarray(np.asarray(W_msg[l], np.float32))
        common[f"ws{l}"] = np.ascontiguousarray(np.asarray(W_self[l], np.float32))
        common[f"we{l}"] = np.ascontiguousarray(np.asarray(W_edge[l], np.float32))
        ewc = np.zeros((128 - DD, D), np.float32)
        ewc[0] = np.asarray(b_msg[l], np.float32)
        ewc[1] = np.asarray(b_edge[l], np.float32)
        ewc[2] = np.asarray(b_self[l], np.float32)
        common[f"ewc{l}"] = ewc
        common[f"gam{l}"] = np.asarray(bn_gamma[l], np.float32).reshape(D, 1)
        common[f"bet{l}"] = np.asarray(bn_beta[l], np.float32).reshape(D, 1)
        g2 = np.asarray(bn_gamma[l], np.float32).reshape(2, 128)
        b2 = np.asarray(bn_beta[l], np.float32).reshape(2, 128)
        common[f"gbrow{l}"] = np.concatenate([g2, b2], axis=0)

    in_maps = [{**common, **pc} for pc in per_core]
    trace = bool(os.environ.get("GNN_TRN_TRACE"))
    res = bass_utils.run_bass_kernel_spmd(
        nc, in_maps, core_ids=list(range(NCORE)), trace=trace)
    if trace:
        global LAST_RESULT
        LAST_RESULT = res
    out = np.concatenate([res.results[c]["out"] for c in range(NCORE)], axis=0)
    return out.reshape(B, S, D).astype(np.float32)


LAST_RESULT = None


# revision 29
# speedup vs baseline: 1.0354x; 1.0354x over previous
"""EntityAggrNet (2-layer GNN message passing) on 8 Trainium2 NeuronCores.

Strategy
--------
Node-parallel sharding: core w owns nodes [w*2048, (w+1)*2048).  Edges are
sorted by src on the host; each core processes the edges whose src lands in
its node range (edge counts per core are within ~1% of E/8 for random edges).

Per layer, per core:
  1. dma_gather x[dst] rows (fp32r, 1KB rows) from a replicated HBM copy of
     the layer input, 2048 edges per gather call.
  2. Segment-sum via one-hot selector matmuls: for each 128-edge chunk,
     selector[p, j] = (src[p] - window_base == j) built with a DVE is_equal
     against an iota row; PE accumulates selector.T @ gathered into a PSUM
     window of 128 nodes.  Mean = PSUM * (1/cnt) on evacuation.
  3. Linearity moves the weight matmuls out of the edge loop:
     mean(x[dst]) @ W_msg.  The edge-feature path collapses to
     Hn @ (emb_table @ W_edge) where Hn[n, d] = count(src=n, feat=d)/cnt[n]
     is a host-computed *index* matrix; biases ride along as extra Hn rows.
  4. Dense phase in feature-major layout (PE transposes), BatchNorm stats
     via a 2KB AllReduce, scale+shift+ReLU fused on the scalar engine.
  5. Layer output is transposed back to node-major and AllGathered (fp32r)
     to become the next layer's gather source.

All matmuls run as float32r (TRN2 full-rate fp32, ~1e-4 rounding).
"""
import os
import sys

if "/opt/trn_rl_repo" not in sys.path:
    sys.path.insert(0, "/opt/trn_rl_repo")

import numpy as np

import concourse.bass as bass  # noqa: F401  (engine types referenced via nc)
import concourse.tile as tile
from concourse import bacc, mybir
from concourse import bass_utils
from concourse.bass_interp import get_hw_module

F32 = mybir.dt.float32
F32R = mybir.dt.float32r
I16 = mybir.dt.int16
BF16 = mybir.dt.bfloat16
ALU = mybir.AluOpType
ACTF = mybir.ActivationFunctionType

EPS = 1e-5
B, S, D = 32, 512, 256
N = B * S                # 16384 nodes
DS, DD = 64, 64          # embedding table: [DS, DD]
L = 2                    # layers
NCORE = 8
NPC = N // NCORE         # 2048 nodes per core
WIN = 128                # nodes per PSUM window
NWIN = NPC // WIN        # 16 windows per core
NWING = N // WIN         # 128 windows globally
PIECE = 512              # edges per dma_gather call (4 chunks)

_CACHE = {}


def _build(nch):
    """Build + schedule + bacc-compile the SPMD program.

    nch: chunks (of 128 edges) per 128-node window, uniform across cores
    (host pads every window to nch*128 edges).
    """
    cap = nch * WIN                  # padded edges per window
    epc = NWIN * cap                 # padded edges per core
    npiece = epc // PIECE            # gather calls per layer
    assert epc % PIECE == 0

    nc = bacc.Bacc("TRN2", target_bir_lowering=False, debug=False,
                   num_devices=NCORE, num_swdge_queues=4)

    # ---- I/O ----
    x0 = nc.dram_tensor("x0", [N, D], BF16, kind="ExternalInput")
    xT0 = nc.dram_tensor("xT0", [D, NPC], F32R, kind="ExternalInput")
    idx_in = nc.dram_tensor("idx_in", [128, epc // 16], I16, kind="ExternalInput")
    srcmod_in = nc.dram_tensor("srcmod_in", [128, NWIN * nch], BF16, kind="ExternalInput")
    recip_in = nc.dram_tensor("recip_in", [128, NWIN], F32, kind="ExternalInput")
    hnt_in = nc.dram_tensor("hnt_in", [128, NPC], F32R, kind="ExternalInput")
    iota_in = nc.dram_tensor("iota_in", [128, 128], BF16, kind="ExternalInput")
    ident_in = nc.dram_tensor("ident_in", [128, 128], F32, kind="ExternalInput")
    wm_in = [nc.dram_tensor(f"wm{l}", [D, D], F32R, kind="ExternalInput") for l in range(L)]
    ws_in = [nc.dram_tensor(f"ws{l}", [D, D], F32R, kind="ExternalInput") for l in range(L)]
    embT_in = nc.dram_tensor("embT", [DD, DS], F32R, kind="ExternalInput")
    we_in = [nc.dram_tensor(f"we{l}", [DD, D], F32R, kind="ExternalInput") for l in range(L)]
    # rows DD..127 of the EW lhsT: [bm, be, bs, zeros...] packed on host
    ewc_in = [nc.dram_tensor(f"ewc{l}", [128 - DD, D], F32R, kind="ExternalInput")
              for l in range(L)]
    gam_in = [nc.dram_tensor(f"gam{l}", [D, 1], F32, kind="ExternalInput") for l in range(L)]
    bet_in = [nc.dram_tensor(f"bet{l}", [D, 1], F32, kind="ExternalInput") for l in range(L)]
    gbrow_in = [nc.dram_tensor(f"gbrow{l}", [4, 128], F32, kind="ExternalInput")
                for l in range(L)]
    out_ext = nc.dram_tensor("out", [NPC, D], F32, kind="ExternalOutput")

    with tile.TileContext(nc) as tc:
        with tc.tile_pool(name="const", bufs=1) as cp, \
             tc.tile_pool(name="gat", bufs=12) as gp, \
             tc.tile_pool(name="selp", bufs=12) as sp, \
             tc.tile_pool(name="xmaj", bufs=1) as xp, \
             tc.tile_pool(name="psE", bufs=4, space="PSUM") as psE, \
             tc.tile_pool(name="psT", bufs=2, space="PSUM") as psT, \
             tc.tile_pool(name="psD", bufs=2, space="PSUM") as psD, \
             tc.tile_pool(name="dram", bufs=1, space="DRAM") as dp:

            # ---- constants into SBUF ----
            NSPLIT = 16 * (PIECE // 16)
            idx_a = cp.tile([128, NSPLIT], I16)
            idx_b = cp.tile([128, epc // 16 - NSPLIT], I16)
            srcmod = cp.tile([128, NWIN * nch], BF16)
            recip = cp.tile([128, NWIN], F32)
            hnt = cp.tile([128, NPC], F32R)
            iota = cp.tile([128, 128], BF16)
            ident = cp.tile([128, 128], F32)
            ones1 = cp.tile([1, 128], F32)
            nc.vector.memset(ones1[:, :], 1.0)
            # split the idx preload so the first gathers start early
            nc.sync.dma_start(out=idx_a[:, :], in_=idx_in[:, 0:NSPLIT])
            nc.sync.dma_start(out=idx_b[:, :], in_=idx_in[:, NSPLIT:])
            nc.sync.dma_start(out=srcmod[:, :], in_=srcmod_in[:, :])
            nc.sync.dma_start(out=recip[:, :], in_=recip_in[:, :])
            nc.sync.dma_start(out=hnt[:, :], in_=hnt_in[:, :])
            nc.sync.dma_start(out=iota[:, :], in_=iota_in[:, :])
            nc.sync.dma_start(out=ident[:, :], in_=ident_in[:, :])

            wm_sb, ws_sb, ew_sb = [], [], []
            embT_sb = cp.tile([DD, DS], F32R)
            nc.sync.dma_start(out=embT_sb[:, :], in_=embT_in[:, :])
            for l in range(L):
                wm = cp.tile([128, 2, D], F32R, name=f"wm_sb{l}")
                ws = cp.tile([128, 2, D], F32R, name=f"ws_sb{l}")
                for kt in range(2):
                    nc.sync.dma_start(out=wm[:, kt, :], in_=wm_in[l][kt * 128:(kt + 1) * 128, :])
                    nc.sync.dma_start(out=ws[:, kt, :], in_=ws_in[l][kt * 128:(kt + 1) * 128, :])
                wm_sb.append(wm)
                ws_sb.append(ws)

                we = cp.tile([DD, D], F32R, name=f"we_sb{l}")
                nc.sync.dma_start(out=we[:, :], in_=we_in[l][:, :])
                psew = psT.tile([DD, D], F32, tag="pst", name=f"psew{l}")
                nc.tensor.matmul(psew[:, :], embT_sb[:, :], we[:, :], start=True, stop=True)
                ew = cp.tile([128, D], F32R, name=f"ew_sb{l}")
                nc.vector.tensor_copy(ew[0:DD, :], psew[:, :])
                nc.sync.dma_start(out=ew[DD:128, :], in_=ewc_in[l][:, :])
                ew_sb.append(ew)

            gb_sb = []  # [128, 2] gamma / beta per layer, packed per feat-half
            for l in range(L):
                gam = cp.tile([128, 2], F32, name=f"gam_sb{l}")
                bet = cp.tile([128, 2], F32, name=f"bet_sb{l}")
                for f in range(2):
                    nc.sync.dma_start(out=gam[:, f:f + 1], in_=gam_in[l][f * 128:(f + 1) * 128, :])
                    nc.sync.dma_start(out=bet[:, f:f + 1], in_=bet_in[l][f * 128:(f + 1) * 128, :])
                gb_sb.append((gam, bet))


            gbrow_sb = []
            for l in range(L):
                gbr = cp.tile([1, 4, 128], F32, name=f"gbrow_sb{l}")
                nc.sync.dma_start(out=gbr[0:1, :, :], in_=gbrow_in[l][:, :])
                gbrow_sb.append(gbr)

            xT_cur = [cp.tile([128, NPC], F32R, name=f"xT0_sb{f}") for f in range(2)]
            for f in range(2):
                nc.sync.dma_start(out=xT_cur[f][:, :], in_=xT0[f * 128:(f + 1) * 128, :])

            # absorb one-time collective setup cost under the edge phase
            warm_sb = cp.tile([128, 1], F32, name="warm_sb")
            nc.vector.memset(warm_sb[:, :], 0.0)
            warm_in = dp.tile([128, 1], F32, name="warm_in")
            warm_out = dp.tile([128, 1], F32, addr_space="Shared", name="warm_out")
            nc.sync.dma_start(out=warm_in[:, :], in_=warm_sb[:, :])
            nc.gpsimd.collective_compute(
                "AllReduce", ALU.add,
                replica_groups=[list(range(NCORE))],
                ins=[warm_in[:, :]], outs=[warm_out[:, :]])
            warm_bk = cp.tile([128, 1], F32, name="warm_bk")
            nc.sync.dma_start(out=warm_bk[:, :], in_=warm_out[:, :])

            xsrc = x0  # gather source (DRAM AP-able handle)

            for l in range(L):
                # ================= edge phase =================
                pieces = [None] * npiece

                def ensure_piece(p, l=l, pieces=pieces):
                    if pieces[p] is not None:
                        return pieces[p]
                    g = gp.tile([128, PIECE // 128, D], BF16, tag="g", name=f"g{l}_{p}")
                    nc.gpsimd.dma_gather(
                        out_ap=g[:, :, :],
                        in_ap=xsrc[:, :],
                        idxs_ap=(idx_a[:, p * (PIECE // 16):(p + 1) * (PIECE // 16)]
                                 if p < 16 else
                                 idx_b[:, (p - 16) * (PIECE // 16):(p - 15) * (PIECE // 16)]),
                        num_idxs=PIECE, num_idxs_reg=PIECE,
                        elem_size=D, single_packet=True,
                        queue_num=p % 4)
                    s = sp.tile([128, PIECE // 128, 128], BF16, tag="s", name=f"s{l}_{p}")
                    npc_ = PIECE // 128
                    nc.vector.tensor_tensor(
                        s[:, :, :],
                        iota[:, :].unsqueeze(1).to_broadcast((128, npc_, 128)),
                        srcmod[:, p * npc_:(p + 1) * npc_].unsqueeze(2)
                              .to_broadcast((128, npc_, 128)),
                        ALU.is_equal)
                    pieces[p] = (g, s)
                    return pieces[p]

                # Fused edge + dense pipeline: windows stream through; after
                # every 4th window the corresponding 512-node dense block,
                # its stat partials, and its node-major transposes fire, so
                # by the time the last edge matmul lands almost everything
                # downstream has already drained.
                msx = xp.tile([128, NWIN, D], F32, tag="msx", name=f"msx{l}")
                preout = [xp.tile([128, NPC], F32, tag=f"pre{f}", name=f"pre{l}_{f}")
                          for f in range(2)]
                xraw = xp.tile([128, NWIN, D], F32, tag="xraw", name=f"xraw{l}")
                redp = cp.tile([128, 16], F32, tag="redp", bufs=2, name=f"redp{l}")
                sqscr = xp.tile([128, 512], F32, tag="sqscr", name=f"sqscr{l}")
                msxTn = [[None] * (NPC // 512) for _ in range(2)]
                for w in range(NWIN):
                    ps = psE.tile([128, D], F32, tag="pse", name=f"pse{l}_{w}")
                    for c in range(nch):
                        gc = w * nch + c
                        g, s = ensure_piece(gc // (PIECE // 128))
                        lc = gc % (PIECE // 128)
                        nc.tensor.matmul(ps[:, :], s[:, lc, :], g[:, lc, :],
                                         start=(c == 0), stop=(c == nch - 1))
                    nc.vector.tensor_scalar(msx[:, w, :], ps[:, :],
                                            recip[:, w:w + 1], None, ALU.mult)
                    nb, wi = w // 4, w % 4
                    for f in range(2):
                        if wi == 0:
                            msxTn[f][nb] = xp.tile([128, 512], F32R, tag=f"msxT{f}",
                                                   bufs=4, name=f"msxT{l}_{f}_{nb}")
                        pt = psT.tile([128, 128], F32, tag="pst", name=f"ptm{l}_{w}_{f}")
                        nc.tensor.transpose(pt[:, :], msx[:, w, f * 128:(f + 1) * 128],
                                            ident[:, :])
                        nc.vector.tensor_copy(msxTn[f][nb][:, wi * 128:(wi + 1) * 128],
                                              pt[:, :])
                    if wi != 3:
                        continue
                    # dense block for this group of 4 windows
                    cols = slice(nb * 512, (nb + 1) * 512)
                    for f in range(2):
                        pd = psD.tile([128, 512], F32, tag="psd", name=f"pd{l}_{f}_{nb}")
                        fo = slice(f * 128, (f + 1) * 128)
                        nc.tensor.matmul(pd[:, :], wm_sb[l][:, 0, fo], msxTn[0][nb][:, :],
                                         start=True, stop=False)
                        nc.tensor.matmul(pd[:, :], wm_sb[l][:, 1, fo], msxTn[1][nb][:, :],
                                         start=False, stop=False)
                        nc.tensor.matmul(pd[:, :], ws_sb[l][:, 0, fo], xT_cur[0][:, cols],
                                         start=False, stop=False)
                        nc.tensor.matmul(pd[:, :], ws_sb[l][:, 1, fo], xT_cur[1][:, cols],
                                         start=False, stop=False)
                        nc.tensor.matmul(pd[:, :], ew_sb[l][:, fo], hnt[:, cols],
                                         start=False, stop=True)
                        # evacuate + free per-block column sums
                        nc.vector.tensor_scalar(preout[f][:, cols], pd[:, :],
                                                1.0, 0.0, ALU.mult, ALU.add,
                                                accum_out=redp[:, f * 4 + nb:f * 4 + nb + 1])
                        # per-block sum of squares on the scalar engine
                        nc.scalar.activation(sqscr[:, :], preout[f][:, cols],
                                             ACTF.Square, bias=0.0, scale=1.0,
                                             accum_out=redp[:, 8 + f * 4 + nb:
                                                            9 + f * 4 + nb])
                    # node-major transposes of this block's pre-BN output
                    for w2 in range(nb * 4, nb * 4 + 4):
                        for f in range(2):
                            pt = psT.tile([128, 128], F32, tag="pst",
                                          name=f"ptx{l}_{w2}_{f}")
                            nc.tensor.transpose(pt[:, :],
                                                preout[f][:, w2 * 128:(w2 + 1) * 128],
                                                ident[:, :])
                            nc.vector.tensor_copy(xraw[:, w2, f * 128:(f + 1) * 128],
                                                  pt[:, :])

                # ================= batchnorm stats =================
                red = cp.tile([128, 4], F32, tag="red", bufs=2, name=f"red{l}")
                for f in range(2):
                    nc.vector.tensor_reduce(red[:, f:f + 1], redp[:, f * 4:(f + 1) * 4],
                                            mybir.AxisListType.X, ALU.add)
                    nc.vector.tensor_reduce(red[:, 2 + f:3 + f],
                                            redp[:, 8 + f * 4:8 + (f + 1) * 4],
                                            mybir.AxisListType.X, ALU.add)

                st_in = dp.tile([128, 4], F32, name=f"st_in{l}")
                st_out = dp.tile([128, 4], F32, addr_space="Shared", name=f"st_out{l}")
                nc.sync.dma_start(out=st_in[:, :], in_=red[:, :])
                nc.gpsimd.collective_compute(
                    "AllReduce", ALU.add,
                    replica_groups=[list(range(NCORE))],
                    ins=[st_in[:, :]], outs=[st_out[:, :]])
                if l < L - 1:
                    red2 = cp.tile([128, 4], F32, tag="red", bufs=2, name=f"red2{l}")
                    nc.sync.dma_start(out=red2[:, :], in_=st_out[:, :])

                    # mu/var -> scale/shift  (all [128, 2])
                    mo = cp.tile([128, 12], F32, tag="mo", bufs=2, name=f"mo{l}")
                    mu, ex2, var, vare, sd, rsq = (mo[:, 0:2], mo[:, 2:4], mo[:, 4:6],
                                                   mo[:, 6:8], mo[:, 8:10], mo[:, 10:12])
                    nc.vector.tensor_scalar(mu, red2[:, 0:2], 1.0 / N, None, ALU.mult)
                    nc.vector.tensor_scalar(ex2, red2[:, 2:4], 1.0 / N, None, ALU.mult)
                    nc.vector.tensor_tensor(var, mu, mu, ALU.mult)
                    nc.vector.tensor_tensor(var, ex2, var, ALU.subtract)
                    nc.vector.tensor_scalar(vare, var, EPS, None, ALU.add)
                    nc.scalar.activation(sd, vare, ACTF.Sqrt, bias=0.0, scale=1.0)
                    nc.vector.reciprocal(rsq, sd)
                    gam, bet = gb_sb[l]
                    sc = cp.tile([128, 4], F32, tag="sc", bufs=2, name=f"sc{l}")
                    scale2, shift2 = sc[:, 0:2], sc[:, 2:4]
                    nc.vector.tensor_tensor(scale2, gam[:, :], rsq, ALU.mult)
                    nc.vector.tensor_tensor(shift2, mu, scale2, ALU.mult)
                    nc.vector.tensor_tensor(shift2, bet[:, :], shift2, ALU.subtract)

                # ===== broadcast scale/shift along partitions (node-major BN) =====
                # row-form stats straight from the AllReduce output in DRAM
                redrow = cp.tile([1, 4, 128], F32, tag="redrow", bufs=2, name=f"redrow{l}")
                nc.scalar.dma_start(out=redrow[0:1, :, :], in_=st_out[:, :].transpose([1, 0]))
                morow = cp.tile([1, 14, 128], F32, tag="morow", bufs=2, name=f"morow{l}")
                mu_r = morow[0:1, 0:2, :]
                ex2_r = morow[0:1, 2:4, :]
                var_r = morow[0:1, 4:6, :]
                vare_r = morow[0:1, 6:8, :]
                sd_r = morow[0:1, 8:10, :]
                rsq_r = morow[0:1, 10:12, :]
                nc.vector.tensor_scalar(mu_r, redrow[0:1, 0:2, :], 1.0 / N, None, ALU.mult)
                nc.vector.tensor_scalar(ex2_r, redrow[0:1, 2:4, :], 1.0 / N, None, ALU.mult)
                nc.vector.tensor_tensor(var_r, mu_r, mu_r, ALU.mult)
                nc.vector.tensor_tensor(var_r, ex2_r, var_r, ALU.subtract)
                nc.vector.tensor_scalar(vare_r, var_r, EPS, None, ALU.add)
                nc.scalar.activation(sd_r, vare_r, ACTF.Sqrt, bias=0.0, scale=1.0)
                nc.vector.reciprocal(rsq_r, sd_r)
                gbrow = gbrow_sb[l]  # [1, 4, 128]: gamma rows 0:2, beta rows 2:4
                scrow = cp.tile([1, 4, 128], F32, tag="scrow", bufs=2, name=f"scrow{l}")
                nc.vector.tensor_tensor(scrow[0:1, 0:2, :], gbrow[0:1, 0:2, :], rsq_r,
                                        ALU.mult)
                nc.vector.tensor_tensor(scrow[0:1, 2:4, :], mu_r, scrow[0:1, 0:2, :],
                                        ALU.mult)
                nc.vector.tensor_tensor(scrow[0:1, 2:4, :], gbrow[0:1, 2:4, :],
                                        scrow[0:1, 2:4, :], ALU.subtract)
                scb = cp.tile([128, 2, D], F32, tag="scb", bufs=2, name=f"scb{l}")
                for j in range(2):  # j=0: scale, j=1: shift
                    pb = psT.tile([128, D], F32, tag="pst", name=f"pb{l}_{j}")
                    nc.tensor.matmul(pb[:, 0:128], ones1[:, :], scrow[0:1, 2 * j, :],
                                     start=True, stop=False)
                    nc.tensor.matmul(pb[:, 128:256], ones1[:, :], scrow[0:1, 2 * j + 1, :],
                                     start=False, stop=True)
                    nc.vector.tensor_copy(scb[:, j, :], pb[:, :])

                # ===== apply BN + ReLU node-major (batched); ship out =====
                if l < L - 1:
                    agi = dp.tile([NPC, D], BF16, name=f"agi{l}")
                    ago = dp.tile([N, D], BF16, addr_space="Shared", name=f"ago{l}")
                tmp = xp.tile([128, NWIN, D], F32, tag="msx", name=f"xtmp{l}")
                xrow_all = xp.tile([128, NWIN, D], F32 if l == L - 1 else BF16,
                                   tag="msx" if l == L - 1 else "xrowall",
                                   name=f"xrow{l}")
                nc.vector.scalar_tensor_tensor(
                    tmp[:, :, :], xraw[:, :, :], 1.0,
                    scb[:, 0, :].unsqueeze(1).to_broadcast((128, NWIN, D)),
                    ALU.mult, ALU.mult)
                nc.vector.tensor_tensor(
                    xraw[:, :, :], tmp[:, :, :],
                    scb[:, 1, :].unsqueeze(1).to_broadcast((128, NWIN, D)), ALU.add)
                nc.vector.tensor_scalar(xrow_all[:, :, :], xraw[:, :, :], 0.0, None,
                                        ALU.max)
                dst = agi if l < L - 1 else out_ext
                dst_ap = dst[:, :].rearrange("(w p) d -> p w d", p=128)
                nc.sync.dma_start(out=dst_ap, in_=xrow_all[:, :, :])

                if l < L - 1:
                    # feat-major post-BN activations for the next layer's self path
                    xnT = [xp.tile([128, NPC], F32R, tag=f"xnT{f}", name=f"xnT{l}_{f}")
                           for f in range(2)]
                    for f in range(2):
                        nc.scalar.activation(xnT[f][:, :], preout[f][:, :], ACTF.Relu,
                                             bias=shift2[:, f:f + 1],
                                             scale=scale2[:, f:f + 1])
                    nc.gpsimd.collective_compute(
                        "AllGather", ALU.bypass,
                        replica_groups=[list(range(NCORE))],
                        ins=[agi[:, :]], outs=[ago[:, :]])
                    xsrc = ago
                    xT_cur = xnT

    nc.compile()
    nc.m = get_hw_module(nc.m)
    return nc


def _preprocess(data, edge, edge_feature):
    """Host-side index preprocessing: sort edges by src, window-pad, build
    count matrices.  Touches only index arrays (+ dtype/layout of data)."""
    src = np.asarray(edge[0], dtype=np.int64)
    dst = np.asarray(edge[1], dtype=np.int64)
    ef = np.asarray(edge_feature, dtype=np.int64)

    order = np.argsort(src, kind="stable")
    src_s = src[order]
    dst_s = dst[order]

    cnt = np.bincount(src, minlength=N)
    recip = (1.0 / np.maximum(cnt, 1)).astype(np.float32)
    H = np.bincount(src * DS + ef, minlength=N * DS).reshape(N, DS)
    Hn = (H * recip[:, None]).astype(np.float32)

    wid = src_s // WIN
    wcnt = np.bincount(wid, minlength=NWING)
    nch = max(int(np.ceil(wcnt.max() / 128)), 1)
    cap = nch * WIN

    wstart = np.zeros(NWING + 1, np.int64)
    np.cumsum(wcnt, out=wstart[1:])
    idx_pad = np.zeros((NWING, cap), np.int16)
    srm_pad = np.full((NWING, cap), -1.0, np.float32)
    for g in range(NWING):
        a, b = wstart[g], wstart[g + 1]
        k = b - a
        idx_pad[g, :k] = dst_s[a:b].astype(np.int16)
        srm_pad[g, :k] = (src_s[a:b] - g * WIN).astype(np.float32)

    per_core = []
    for w in range(NCORE):
        gsl = slice(w * NWIN, (w + 1) * NWIN)
        nsl = slice(w * NPC, (w + 1) * NPC)
        flat_idx = idx_pad[gsl].reshape(-1)           # [NWIN*cap]
        idx_tile = np.tile(flat_idx.reshape(-1, 16).T, (8, 1)).astype(np.int16)
        srcmod = srm_pad[gsl].reshape(-1, 128).T.copy()      # [128, NWIN*nch]
        recip_sw = recip[nsl].reshape(NWIN, 128).T.copy()    # [128, NWIN]
        hnt = np.zeros((128, NPC), np.float32)
        hnt[:DS, :] = Hn[nsl].T
        nz = (cnt[nsl] > 0).astype(np.float32)
        hnt[DS, :] = nz
        hnt[DS + 1, :] = nz
        hnt[DS + 2, :] = 1.0
        xT0 = np.ascontiguousarray(
            data.reshape(N, D)[nsl].T.astype(np.float32))
        import ml_dtypes as _md
        per_core.append(dict(idx_in=idx_tile, srcmod_in=srcmod.astype(_md.bfloat16),
                             recip_in=recip_sw, hnt_in=hnt, xT0=xT0))
    return nch, per_core


def kernel(data, emb_table, W_msg, b_msg, W_self, b_self, W_edge, b_edge,
           bn_gamma, bn_beta, edge, edge_feature):
    data = np.asarray(data)
    nch, per_core = _preprocess(data, np.asarray(edge), np.asarray(edge_feature))

    if nch not in _CACHE:
        _CACHE[nch] = _build(nch)
    nc = _CACHE[nch]

    import ml_dtypes
    x0 = np.ascontiguousarray(data.reshape(N, D).astype(ml_dtypes.bfloat16))
    iota = np.broadcast_to(np.arange(128), (128, 128)).astype(ml_dtypes.bfloat16)
    ident = np.eye(128, dtype=np.float32)
    common = {
        "x0": x0, "iota_in": iota, "ident_in": ident,
        "embT": np.ascontiguousarray(np.asarray(emb_table, np.float32).T),
    }
    for l in range(L):
        common[f"wm{l}"] = np.ascontiguousarray(np.asarray(W_msg[l], np.float32))
        common[f"ws{l}"] = np.ascontiguousarray(np.asarray(W_self[l], np.float32))
        common[f"we{l}"] = np.ascontiguousarray(np.asarray(W_edge[l], np.float32))
        ewc = np.zeros((128 - DD, D), np.float32)
        ewc[0] = np.asarray(b_msg[l], np.float32)
        ewc[1] = np.asarray(b_edge[l], np.float32)
        ewc[2] = np.asarray(b_self[l], np.float32)
        common[f"ewc{l}"] = ewc
        common[f"gam{l}"] = np.asarray(bn_gamma[l], np.float32).reshape(D, 1)
        common[f"bet{l}"] = np.asarray(bn_beta[l], np.float32).reshape(D, 1)
        g2 = np.asarray(bn_gamma[l], np.float32).reshape(2, 128)
        b2 = np.asarray(bn_beta[l], np.float32).reshape(2, 128)
        common[f"gbrow{l}"] = np.concatenate([g2, b2], axis=0)

    in_maps = [{**common, **pc} for pc in per_core]
    trace = bool(os.environ.get("GNN_TRN_TRACE"))
    res = bass_utils.run_bass_kernel_spmd(
        nc, in_maps, core_ids=list(range(NCORE)), trace=trace)
    if trace:
        global LAST_RESULT
        LAST_RESULT = res
    out = np.concatenate([res.results[c]["out"] for c in range(NCORE)], axis=0)
    return out.reshape(B, S, D).astype(np.float32)


LAST_RESULT = None
